# revision 30
# baseline (speedup 1.0000x reference)
"""Trainium2 Bass kernel for nn_MLPMHA (sparse_attention / squared-ReLU MLP-MHA).

Reference computation (B=4, T=2048, C=1024, QH=4, D=256, S=4C=4096):
    x   = layernorm(residual) * g + b
    q_h = x[:, h*D:(h+1)*D]                     per head h
    k   = w_fc.reshape(S, D)                    keys   (shared across heads)
    v   = w_proj.T.reshape(S, D)                values (shared across heads)
    out = residual + concat_h( relu(q_h @ k.T)^2 @ v )

Sharding: pure data parallel over the 8192 = B*T token rows; each of the 8
cores processes 1024 rows with full weights resident in SBUF.

v2 design (vs the fp32r v1 baseline; 443us recorded -> ~301us measured):
  * ln_g is folded into w_fc on the host (wfcT_eff = (w_fc * g).T); a nonzero
    ln_b adds a per-key score bias sb[h,s] (host precomputed) applied in the
    relu stage. With that, x_hat needs no per-channel affine after the
    transpose into [channel, token] layout.
  * All matmul operands are bf16 (same 1 cycle/row as fp32r on the PE, but
    half the SBUF/DMA traffic and 2x faster PE transposes; measured
    end-to-end rel err ~4e-3 vs the 2e-2 tolerance).
  * relu^2 runs as two ops (relu: PSUM->SBUF bf16, square: bf16 SBUF at 2x
    DVE rate), engines alternating per block (pattern "ZW"), so psA banks
    recycle fast and neither ACT nor DVE becomes the gate.
  * Phase B is split into 8 sub-streams (head x token-half); the first only
    needs row tiles 0-3 transposed, so the PE starts ~9us in instead of
    ~31us. Tiles 4-7 run their LayerNorm mid-sub-stream-0 and are transposed
    by the DMA XBAR (dma_start_transpose, exact for 2-byte dtypes) on the
    then-idle SP ring; tiles 0-3 use PE transposes (PE is idle at the head).
  * The mm1->relu^2->mm2 chain is software-pipelined with a global pending
    queue that crosses sub-stream boundaries (lookahead blocks); epilogues
    (PSUM -> bf16 -> DMA-transpose -> DVE/Pool add into residual-initialised
    out_sb -> per-column DMA store) are emitted epi_at (> lookahead!) blocks
    into the next sub-stream and never touch the PE.
  * Weight/residual DMAs are issued in consumption order so the first matmuls
    wait only on the first ~1.5MB, not the full 8MB of inputs.

Hardware-measured notes: matmul instruction streams cost ~52ns/mm extra at
kernel-scale loop bodies (instruction fetch) plus ~100ns per cross-engine
semaphore wait; deep buffer rotation (psA=4, pt/rl pools) recovers most of
the latter. A 1024-wide matmul output is illegal (s3d3_mm_num_elements: one
PSUM bank max). GPSIMD/Pool has no PSUM port, and scalar_tensor_tensor may
read at most one PSUM operand.
"""

import numpy as np

import concourse.bass as bass
import concourse.tile as tile
from concourse import mybir, bacc
from concourse.bass_utils import run_bass_kernel_spmd

P = 128
C = 1024
D = 256
QH = 4
NCC = 4          # column chunks of w_fc (S = NCC * C kv entries)
N_CORES = 8
ROWS = 1024      # token rows per core (8192 / 8)
NT = ROWS // P   # 8 row tiles per core
EPS = 1e-5

F32 = mybir.dt.float32
BF16 = mybir.dt.bfloat16

_NC_CACHE = {}

CONFIG = {
    "lookahead": 4,        # mm2 software-pipeline depth behind mm1
    "relu_pat": "ZW",        # relu^2 engine pattern, cycled per half-block:
                             # Z: ACT relu + DVE square   Y: ACT relu + Pool sq
                             # V: DVE relu + Pool square  W: DVE relu + ACT sq
    "pools": (4, 4, 0),    # psA, psO bufs (psT=0: transposes share psA)
    "pt_bufs": 7,          # pT half-tiles in flight
    "rl_bufs": 5,
    "pa2_dma": True,       # transpose tiles 4-7 via the SP DMA ring (idle
                           # mid-stream) instead of PE
    "r45_early": False,    # interleave r4/r5 into the wproj chunk stream
    "epi_at": 6,           # block in sub-stream k where sub-stream k-1's
                           # epilogue is emitted; MUST be > lookahead, or the
                           # epilogue reads po before its last mm2s are
                           # emitted
    "pa2_at": (10, 14, 18, 22),  # blocks of sub-stream 0 where row tiles
                                 # 4-7 run their phase A
}


def _ln_tile(nc, work, resid_sb, tt, eps_t, rep):
    """LayerNorm stats for row tile tt; returns bf16 x_hat tile."""
    stats = work.tile([P, 2, 6], F32, name=f"st_{rep}_{tt}", tag="stats")
    nc.vector.bn_stats(stats[:, 0, :], resid_sb[:, tt, 0:512])
    nc.vector.bn_stats(stats[:, 1, :], resid_sb[:, tt, 512:1024])
    mv = work.tile([P, 2], F32, name=f"mv_{rep}_{tt}", tag="mv")
    nc.vector.bn_aggr(mv[:], stats[:])
    nc.scalar.activation(mv[:, 1:2], mv[:, 1:2],
                         mybir.ActivationFunctionType.Sqrt,
                         bias=eps_t[:], scale=1.0)
    nc.vector.reciprocal(mv[:, 1:2], mv[:, 1:2])
    nmr = work.tile([P, 1], F32, name=f"nmr_{rep}_{tt}", tag="nmr")
    nc.vector.tensor_scalar(out=nmr[:], in0=mv[:, 0:1],
                            scalar1=mv[:, 1:2], scalar2=-1.0,
                            op0=mybir.AluOpType.mult,
                            op1=mybir.AluOpType.mult)
    xn = work.tile([P, C], BF16, name=f"xn_{rep}_{tt}", tag="xn")
    nc.scalar.activation(xn[:], resid_sb[:, tt, :],
                         mybir.ActivationFunctionType.Identity,
                         bias=nmr[:], scale=mv[:, 1:2])
    return xn


def _phase_a_tile(nc, work, psT, resid_sb, out_sb, xT_sb, ident_bf, eps_t,
                  tt, rep, psT_is_psA=False, use_dma=False):
    xn = _ln_tile(nc, work, resid_sb, tt, eps_t, rep)
    if use_dma:
        # XBAR transpose on the SP hwdge ring (idle mid-stream):
        # xT_sb[p, o, tt*P+t] = xn[t, o*P+p]
        nc.sync.dma_start_transpose(xT_sb[:, :, tt * P:(tt + 1) * P], xn[:])
    else:
        # PE transpose in bf16 (1 cycle/row): with ln_g folded into the
        # weights there is no affine, just a PSUM copy-back on ACT.
        for och in range(8):
            _ptag = "psa" if psT_is_psA else "pst"
            pst = psT.tile([P, P], BF16, name=f"psx_{rep}_{tt}_{och}",
                           tag=_ptag)
            nc.tensor.transpose(pst[:], xn[:, och * P:(och + 1) * P],
                                ident_bf[:])
            nc.scalar.activation(xT_sb[:, och, tt * P:(tt + 1) * P], pst[:],
                                 mybir.ActivationFunctionType.Identity)
    # out starts as the residual; head outputs accumulate into it
    nc.gpsimd.tensor_copy(out=out_sb[:, tt, :], in_=resid_sb[:, tt, :])


def _build_body(tc, resid, wfcT, wprojT, sbias, out, reps, variant, has_bias):  # noqa: C901
    nc = tc.nc
    import contextlib
    ctx = contextlib.ExitStack()
    with ctx:
        singles = ctx.enter_context(tc.tile_pool(name="singles", bufs=1))
        work = ctx.enter_context(tc.tile_pool(name="work", bufs=3))
        rlpool = ctx.enter_context(tc.tile_pool(name="rlpool",
                                                bufs=CONFIG["rl_bufs"]))
        ptpool = ctx.enter_context(tc.tile_pool(name="ptpool",
                                                bufs=CONFIG["pt_bufs"]))
        psA = ctx.enter_context(tc.tile_pool(name="psA", bufs=CONFIG["pools"][0],
                                             space="PSUM"))
        psO = ctx.enter_context(tc.tile_pool(name="psO", bufs=CONFIG["pools"][1],
                                             space="PSUM"))
        if CONFIG["pools"][2]:
            psT = ctx.enter_context(tc.tile_pool(name="psT",
                                                 bufs=CONFIG["pools"][2],
                                                 space="PSUM"))
        else:
            psT = psA  # transposes rotate through the psA slots

        # ---- resident tensors, DMA'd in consumption order ----------------
        wfcT_sb = singles.tile([P, 8, C], BF16)
        wprojT_sb = singles.tile([P, 8, C], BF16)
        xT_sb = singles.tile([P, 8, ROWS], BF16)
        resid_sb = singles.tile([P, NT, C], F32)
        out_sb = singles.tile([P, NT, C], F32)
        eps_t = singles.tile([P, 1], F32)
        nc.vector.memset(eps_t[:], EPS)
        ident_bf = singles.tile([P, P], BF16)
        from concourse.masks import make_identity
        make_identity(nc, ident_bf[:])
        sb_sb = None
        if has_bias:
            sb_sb = singles.tile([P, 128], F32)
            nc.sync.dma_start(sb_sb[:], sbias.rearrange("(k p) -> p k", p=P))

        def dma_wfc(o):
            nc.sync.dma_start(wfcT_sb[:, o, :], wfcT[o * P:(o + 1) * P, :])

        def dma_wproj(o):
            nc.sync.dma_start(wprojT_sb[:, o, :], wprojT[o * P:(o + 1) * P, :])

        def dma_resid(tt):
            nc.sync.dma_start(resid_sb[:, tt, :], resid[tt * P:(tt + 1) * P, :])

        # SP-queue issue order = consumption order: the first sub-stream
        # needs row tiles 0-3, wfc chunks progressively and wproj from block
        # ~4; tiles 4-7 arrive while sub-stream 0 executes.
        for tt in range(4):
            dma_resid(tt)
        dma_wfc(0); dma_wfc(1)
        dma_wfc(2); dma_wfc(3)
        if CONFIG["r45_early"]:
            for o in range(4):
                dma_wproj(o)
            dma_resid(4)
            dma_wproj(4); dma_wproj(5)
            dma_resid(5)
            dma_wproj(6); dma_wproj(7)
        else:
            for o in range(8):
                dma_wproj(o)
            dma_resid(4); dma_resid(5)
        dma_wfc(4); dma_wfc(5)
        dma_resid(6); dma_resid(7)
        dma_wfc(6); dma_wfc(7)

        pT_dummy = None
        if variant != 'full':
            pT_dummy = singles.tile([P, 512], BF16)
            nc.sync.dma_start(pT_dummy[:], wfcT[0:P, 0:512])
            nc.sync.dma_start(xT_sb[:], wfcT.rearrange("(o p) i -> p o i", p=P)
                              [:, :, 0:ROWS])
            nc.vector.memset(out_sb[:], 0.0)

        if reps == 1:
            _phase_abc(nc, tc, work, rlpool, ptpool, psA, psO, psT, ident_bf,
                       resid, out, wfcT_sb, wprojT_sb, xT_sb, resid_sb,
                       out_sb, eps_t, sb_sb, 0, variant, pT_dummy, has_bias)
        else:
            hint = (mybir.EngineType.PE, mybir.EngineType.Activation,
                    mybir.EngineType.DVE, mybir.EngineType.SP,
                    mybir.EngineType.Pool)
            with tc.For_i(0, reps, 1, hint_engines=hint):
                _phase_abc(nc, tc, work, rlpool, ptpool, psA, psO, psT,
                           ident_bf, resid, out, wfcT_sb, wprojT_sb, xT_sb,
                           resid_sb, out_sb, eps_t, sb_sb, 0, variant,
                           pT_dummy, has_bias)


def _phase_abc(nc, tc, work, rlpool, ptpool, psA, psO, psT, ident_bf,
               resid, out, wfcT_sb, wprojT_sb, xT_sb, resid_sb, out_sb,
               eps_t, sb_sb, rep, variant, pT_dummy, has_bias):
    full = variant == 'full'
    pat = CONFIG["relu_pat"]
    lookahead = CONFIG["lookahead"]
    blk_idx = [0]

    # sub-streams in emission order: (head, token-half)
    streams = [(h, tch) for h in range(QH) for tch in range(2)]
    po = {}      # (h, tch) -> [po_dd0, po_dd1]

    def relu_sq(ps, hb, h, cc, ich):
        """pT = relu(ps + bias)^2 in bf16, engines per pattern schedule."""
        kind = pat[hb % len(pat)]
        rl = rlpool.tile([P, 512], BF16, name=f"rl_{rep}_{hb}", tag="rl")
        pT = ptpool.tile([P, 512], BF16, name=f"pT_{rep}_{hb}", tag="pT")
        if has_bias:
            bias = sb_sb[:, h * 32 + cc * 8 + ich:h * 32 + cc * 8 + ich + 1]
            nc.scalar.activation(rl[:], ps[:],
                                 mybir.ActivationFunctionType.Relu,
                                 bias=bias, scale=1.0)
            if kind in ('Z',):
                nc.vector.tensor_mul(out=pT[:], in0=rl[:], in1=rl[:])
            elif kind in ('Y', 'V'):
                nc.gpsimd.tensor_mul(out=pT[:], in0=rl[:], in1=rl[:])
            else:
                nc.scalar.activation(pT[:], rl[:],
                                     mybir.ActivationFunctionType.Square)
            return pT
        if kind == 'Z':
            nc.scalar.activation(rl[:], ps[:],
                                 mybir.ActivationFunctionType.Relu)
            nc.vector.tensor_mul(out=pT[:], in0=rl[:], in1=rl[:])
        elif kind == 'Y':
            nc.scalar.activation(rl[:], ps[:],
                                 mybir.ActivationFunctionType.Relu)
            nc.gpsimd.tensor_mul(out=pT[:], in0=rl[:], in1=rl[:])
        elif kind == 'V':
            nc.vector.tensor_scalar_max(out=rl[:], in0=ps[:], scalar1=0.0)
            nc.gpsimd.tensor_mul(out=pT[:], in0=rl[:], in1=rl[:])
        else:  # W
            nc.vector.tensor_scalar_max(out=rl[:], in0=ps[:], scalar1=0.0)
            nc.scalar.activation(pT[:], rl[:],
                                 mybir.ActivationFunctionType.Square)
        return pT

    def mm1(h, tch, cc, ich, hb):
        ps = psA.tile([P, 512], F32, name=f"psa_{rep}_{hb}", tag="psa")
        tsl = slice(tch * 512, (tch + 1) * 512)
        isl = slice(ich * P, (ich + 1) * P)
        nc.tensor.matmul(ps[:], wfcT_sb[:, cc * 2 + 0, isl],
                         xT_sb[:, h * 2 + 0, tsl], start=True, stop=False)
        nc.tensor.matmul(ps[:], wfcT_sb[:, cc * 2 + 1, isl],
                         xT_sb[:, h * 2 + 1, tsl], start=False, stop=True)
        return ps

    def mm2(h, tch, cc, ich, pT):
        first = (cc == 0 and ich == 0)
        last = (cc == NCC - 1 and ich == 7)
        for dd in range(2):
            wsl = slice(cc * D + dd * P, cc * D + (dd + 1) * P)
            nc.tensor.matmul(po[(h, tch)][dd][:],
                             wprojT_sb[:, ich, wsl], pT[:],
                             start=first, stop=last)

    def epilogue(h, tch):
        if variant not in ('full', 'b_only'):
            return
        for dd in range(2):
            oc = work.tile([P, 512], BF16, name=f"oc_{rep}_{h}_{tch}_{dd}",
                           tag="oc")
            nc.scalar.activation(oc[:], po[(h, tch)][dd][:],
                                 mybir.ActivationFunctionType.Identity)
            ot = work.tile([P, 4, P], BF16, name=f"ot_{rep}_{h}_{tch}_{dd}",
                           tag="ot")
            nc.sync.dma_start_transpose(ot[:], oc[:])
            csl = slice(h * D + dd * P, h * D + (dd + 1) * P)
            # alternate add engines so the two halves run in parallel
            eng = nc.vector if dd == 0 else nc.gpsimd
            eng.tensor_add(out=out_sb[:, tch * 4:(tch + 1) * 4, csl],
                           in0=out_sb[:, tch * 4:(tch + 1) * 4, csl],
                           in1=ot[:])
            for k in range(4):
                tt = tch * 4 + k
                nc.sync.dma_start(out[tt * P:(tt + 1) * P, csl],
                                  out_sb[:, tt, csl])

    # ---- emission ----------------------------------------------------------
    if full:
        for tt in range(4):
            _phase_a_tile(nc, work, psT, resid_sb, out_sb, xT_sb, ident_bf,
                          eps_t, tt, rep, psT is psA)

    pending = []       # (h, tch, cc, ich, pT) awaiting mm2
    prev_stream = [None]

    for si, (h, tch) in enumerate(streams):
        po[(h, tch)] = [psO.tile([P, 512], F32,
                                 name=f"po_{rep}_{h}_{tch}_{dd}", tag="po")
                        for dd in range(2)]
        for cc in range(NCC):
            for ich in range(8):
                hb = blk_idx[0]
                blk_idx[0] += 1
                nblk = cc * 8 + ich
                if nblk == CONFIG["epi_at"] and prev_stream[0] is not None:
                    epilogue(*prev_stream[0])
                    prev_stream[0] = None
                if si == 0 and full and nblk in CONFIG["pa2_at"]:
                    tt = 4 + CONFIG["pa2_at"].index(nblk)
                    _phase_a_tile(nc, work, psT, resid_sb, out_sb, xT_sb,
                                  ident_bf, eps_t, tt, rep, psT is psA,
                                  use_dma=CONFIG["pa2_dma"])
                ps = mm1(h, tch, cc, ich, hb)
                if variant == 'mm1_only':
                    continue
                if variant == 'mm_only':
                    mm2(h, tch, cc, ich, pT_dummy)
                    continue
                pT = relu_sq(ps, hb, h, cc, ich)
                pending.append((h, tch, cc, ich, pT))
                if len(pending) > lookahead:
                    a = pending.pop(0)
                    mm2(a[0], a[1], a[2], a[3], a[4])
        # the pipeline continues across sub-streams: this stream's last mm2s
        # drain behind the next stream's mm1s, and its epilogue (ACT/DMA/Pool
        # only) is emitted epi_at blocks into the next stream
        prev_stream[0] = (h, tch)
    for a in pending:
        mm2(a[0], a[1], a[2], a[3], a[4])
    if prev_stream[0] is not None:
        epilogue(*prev_stream[0])


def build_nc(reps=1, variant='full', has_bias=False):
    key = (reps, variant, has_bias, str(sorted(CONFIG.items())))
    if key in _NC_CACHE:
        return _NC_CACHE[key]
    nc = bacc.Bacc("TRN2", target_bir_lowering=False, debug=False,
                   num_devices=N_CORES)
    resid = nc.dram_tensor("residual", [ROWS, C], F32, kind="ExternalInput").ap()
    wfcT = nc.dram_tensor("w_fcT", [C, C], BF16, kind="ExternalInput").ap()
    wprojT = nc.dram_tensor("w_projT", [C, C], BF16, kind="ExternalInput").ap()
    sbias = nc.dram_tensor("s_bias", [128 * P], F32, kind="ExternalInput").ap() \
        if True else None
    out = nc.dram_tensor("out", [ROWS, C], F32, kind="ExternalOutput").ap()
    with tile.TileContext(nc) as tc:
        _build_body(tc, resid, wfcT, wprojT, sbias, out, reps, variant,
                    has_bias)
    nc.compile()
    _NC_CACHE[key] = nc
    return nc


def _in_maps(residual, w_fc, w_proj, ln_g, ln_b):
    import ml_dtypes
    resid2d = np.ascontiguousarray(residual.reshape(-1, C))
    # fold ln_g into the keys: scores use x_hat * g as the query
    wfcT = np.ascontiguousarray((w_fc * np.asarray(ln_g)[None, :]).T
                                ).astype(ml_dtypes.bfloat16)
    wprojT = np.ascontiguousarray(w_proj.T).astype(ml_dtypes.bfloat16)
    # per-key score bias from ln_b: sb[h, s=(i,cc)] = sum_d b[h*D+d] k[s,d]
    ln_b = np.asarray(ln_b)
    k = w_fc.reshape(-1, D)                           # (S, D), s = i*4+cc
    sb = ln_b.reshape(QH, D) @ k.T                    # (QH, S)
    # device layout: sb_dev[p, h*32+cc*8+ich] with i = ich*128+p
    s_idx = (np.arange(4096).reshape(-1, 4))          # [i, cc] -> s
    sb_dev = np.zeros((P, 128), np.float32)
    for h in range(QH):
        for cc in range(NCC):
            for ich in range(8):
                i = np.arange(ich * P, (ich + 1) * P)
                sb_dev[:, h * 32 + cc * 8 + ich] = sb[h, s_idx[i, cc]]
    sb_flat = np.ascontiguousarray(sb_dev.T.reshape(-1))  # (k p) -> p k
    return [
        {"residual": resid2d[i * ROWS:(i + 1) * ROWS],
         "w_fcT": wfcT, "w_projT": wprojT, "s_bias": sb_flat}
        for i in range(N_CORES)
    ]


def run_on_cores(inputs, reps=1):
    has_bias = bool(np.any(np.asarray(inputs["ln_b"])))
    nc = build_nc(reps, 'full', has_bias)
    in_maps = _in_maps(**inputs)
    return run_bass_kernel_spmd(nc, in_maps, core_ids=list(range(N_CORES)))


def kernel(residual, w_fc, w_proj, ln_g, ln_b):
    B, T, Cx = residual.shape
    res = run_on_cores(dict(residual=residual, w_fc=w_fc, w_proj=w_proj,
                            ln_g=ln_g, ln_b=ln_b))
    out = np.concatenate([r["out"] for r in res.results], axis=0)
    return out.reshape(B, T, Cx).astype(np.float32)


# revision 33
# speedup vs baseline: 1.0048x; 1.0048x over previous
"""Trainium2 Bass kernel for nn_MLPMHA (sparse_attention / squared-ReLU MLP-MHA).

Reference computation (B=4, T=2048, C=1024, QH=4, D=256, S=4C=4096):
    x   = layernorm(residual) * g + b
    q_h = x[:, h*D:(h+1)*D]                     per head h
    k   = w_fc.reshape(S, D)                    keys   (shared across heads)
    v   = w_proj.T.reshape(S, D)                values (shared across heads)
    out = residual + concat_h( relu(q_h @ k.T)^2 @ v )

Sharding: pure data parallel over the 8192 = B*T token rows; each of the 8
cores processes 1024 rows with full weights resident in SBUF.

v2 design (vs the fp32r v1 baseline; 443us recorded -> ~301us measured):
  * ln_g is folded into w_fc on the host (wfcT_eff = (w_fc * g).T); a nonzero
    ln_b adds a per-key score bias sb[h,s] (host precomputed) applied in the
    relu stage. With that, x_hat needs no per-channel affine after the
    transpose into [channel, token] layout.
  * All matmul operands are bf16 (same 1 cycle/row as fp32r on the PE, but
    half the SBUF/DMA traffic and 2x faster PE transposes; measured
    end-to-end rel err ~4e-3 vs the 2e-2 tolerance).
  * relu^2 runs as two ops (relu: PSUM->SBUF bf16, square: bf16 SBUF at 2x
    DVE rate), engines alternating per block (pattern "ZW"), so psA banks
    recycle fast and neither ACT nor DVE becomes the gate.
  * Phase B is split into 8 sub-streams (head x token-half); the first only
    needs row tiles 0-3 transposed, so the PE starts ~9us in instead of
    ~31us. Tiles 4-7 run their LayerNorm mid-sub-stream-0 and are transposed
    by the DMA XBAR (dma_start_transpose, exact for 2-byte dtypes) on the
    then-idle SP ring; tiles 0-3 use PE transposes (PE is idle at the head).
  * The mm1->relu^2->mm2 chain is software-pipelined with a global pending
    queue that crosses sub-stream boundaries (lookahead blocks); epilogues
    (PSUM -> bf16 -> DMA-transpose -> DVE/Pool add into residual-initialised
    out_sb -> per-column DMA store) are emitted epi_at (> lookahead!) blocks
    into the next sub-stream and never touch the PE.
  * Weight/residual DMAs are issued in consumption order so the first matmuls
    wait only on the first ~1.5MB, not the full 8MB of inputs.

Hardware-measured notes: matmul instruction streams cost ~52ns/mm extra at
kernel-scale loop bodies (instruction fetch) plus ~100ns per cross-engine
semaphore wait; deep buffer rotation (psA=4, pt/rl pools) recovers most of
the latter. A 1024-wide matmul output is illegal (s3d3_mm_num_elements: one
PSUM bank max). GPSIMD/Pool has no PSUM port, and scalar_tensor_tensor may
read at most one PSUM operand.
"""

import numpy as np

import concourse.bass as bass
import concourse.tile as tile
from concourse import mybir, bacc
from concourse.bass_utils import run_bass_kernel_spmd

P = 128
C = 1024
D = 256
QH = 4
NCC = 4          # column chunks of w_fc (S = NCC * C kv entries)
N_CORES = 8
ROWS = 1024      # token rows per core (8192 / 8)
NT = ROWS // P   # 8 row tiles per core
EPS = 1e-5

F32 = mybir.dt.float32
BF16 = mybir.dt.bfloat16

_NC_CACHE = {}

CONFIG = {
    "lookahead": 3,        # mm2 software-pipeline depth behind mm1
    "relu_pat": "S",         # relu^2 engine pattern, cycled per half-block:
                             # S: tile split in half, ACT owns relu+square of
                             #    one half and DVE the other (fastest)
                             # Z: ACT relu + DVE square   Y: ACT relu + Pool sq
                             # V: DVE relu + Pool square  W: DVE relu + ACT sq
    "pools": (4, 4, 0),    # psA, psO bufs (psT=0: transposes share psA)
    "pt_bufs": 7,          # pT half-tiles in flight
    "rl_bufs": 5,
    "pa2_dma": True,       # transpose tiles 4-7 via the SP DMA ring (idle
                           # mid-stream) instead of PE
    "r45_early": False,    # interleave r4/r5 into the wproj chunk stream
    "epi_at": 5,           # block in sub-stream k where sub-stream k-1's
                           # epilogue is emitted; MUST be > lookahead, or the
                           # epilogue reads po before its last mm2s are
                           # emitted
    "pa2_at": (10, 14, 18, 22),  # blocks of sub-stream 0 where row tiles
                                 # 4-7 run their phase A
}


def _ln_tile(nc, work, resid_sb, tt, eps_t, rep):
    """LayerNorm stats for row tile tt; returns bf16 x_hat tile."""
    stats = work.tile([P, 2, 6], F32, name=f"st_{rep}_{tt}", tag="stats")
    nc.vector.bn_stats(stats[:, 0, :], resid_sb[:, tt, 0:512])
    nc.vector.bn_stats(stats[:, 1, :], resid_sb[:, tt, 512:1024])
    mv = work.tile([P, 2], F32, name=f"mv_{rep}_{tt}", tag="mv")
    nc.vector.bn_aggr(mv[:], stats[:])
    nc.scalar.activation(mv[:, 1:2], mv[:, 1:2],
                         mybir.ActivationFunctionType.Sqrt,
                         bias=eps_t[:], scale=1.0)
    nc.vector.reciprocal(mv[:, 1:2], mv[:, 1:2])
    nmr = work.tile([P, 1], F32, name=f"nmr_{rep}_{tt}", tag="nmr")
    nc.vector.tensor_scalar(out=nmr[:], in0=mv[:, 0:1],
                            scalar1=mv[:, 1:2], scalar2=-1.0,
                            op0=mybir.AluOpType.mult,
                            op1=mybir.AluOpType.mult)
    xn = work.tile([P, C], BF16, name=f"xn_{rep}_{tt}", tag="xn")
    nc.scalar.activation(xn[:], resid_sb[:, tt, :],
                         mybir.ActivationFunctionType.Identity,
                         bias=nmr[:], scale=mv[:, 1:2])
    return xn


def _phase_a_tile(nc, work, psT, resid_sb, out_sb, xT_sb, ident_bf, eps_t,
                  tt, rep, psT_is_psA=False, use_dma=False):
    xn = _ln_tile(nc, work, resid_sb, tt, eps_t, rep)
    if use_dma:
        # XBAR transpose on the SP hwdge ring (idle mid-stream):
        # xT_sb[p, o, tt*P+t] = xn[t, o*P+p]
        nc.sync.dma_start_transpose(xT_sb[:, :, tt * P:(tt + 1) * P], xn[:])
    else:
        # PE transpose in bf16 (1 cycle/row): with ln_g folded into the
        # weights there is no affine, just a PSUM copy-back on ACT.
        for och in range(8):
            _ptag = "psa" if psT_is_psA else "pst"
            pst = psT.tile([P, P], BF16, name=f"psx_{rep}_{tt}_{och}",
                           tag=_ptag)
            nc.tensor.transpose(pst[:], xn[:, och * P:(och + 1) * P],
                                ident_bf[:])
            nc.scalar.activation(xT_sb[:, och, tt * P:(tt + 1) * P], pst[:],
                                 mybir.ActivationFunctionType.Identity)
    # out starts as the residual; head outputs accumulate into it
    nc.gpsimd.tensor_copy(out=out_sb[:, tt, :], in_=resid_sb[:, tt, :])


def _build_body(tc, resid, wfcT, wprojT, sbias, out, reps, variant, has_bias):  # noqa: C901
    nc = tc.nc
    import contextlib
    ctx = contextlib.ExitStack()
    with ctx:
        singles = ctx.enter_context(tc.tile_pool(name="singles", bufs=1))
        work = ctx.enter_context(tc.tile_pool(name="work", bufs=3))
        rlpool = ctx.enter_context(tc.tile_pool(name="rlpool",
                                                bufs=CONFIG["rl_bufs"]))
        ptpool = ctx.enter_context(tc.tile_pool(name="ptpool",
                                                bufs=CONFIG["pt_bufs"]))
        psA = ctx.enter_context(tc.tile_pool(name="psA", bufs=CONFIG["pools"][0],
                                             space="PSUM"))
        psO = ctx.enter_context(tc.tile_pool(name="psO", bufs=CONFIG["pools"][1],
                                             space="PSUM"))
        if CONFIG["pools"][2]:
            psT = ctx.enter_context(tc.tile_pool(name="psT",
                                                 bufs=CONFIG["pools"][2],
                                                 space="PSUM"))
        else:
            psT = psA  # transposes rotate through the psA slots

        # ---- resident tensors, DMA'd in consumption order ----------------
        wfcT_sb = singles.tile([P, 8, C], BF16)
        wprojT_sb = singles.tile([P, 8, C], BF16)
        xT_sb = singles.tile([P, 8, ROWS], BF16)
        resid_sb = singles.tile([P, NT, C], F32)
        out_sb = singles.tile([P, NT, C], F32)
        eps_t = singles.tile([P, 1], F32)
        nc.vector.memset(eps_t[:], EPS)
        ident_bf = singles.tile([P, P], BF16)
        from concourse.masks import make_identity
        make_identity(nc, ident_bf[:])
        sb_sb = None
        if has_bias:
            sb_sb = singles.tile([P, 128], F32)
            nc.sync.dma_start(sb_sb[:], sbias.rearrange("(k p) -> p k", p=P))

        def dma_wfc(o):
            nc.sync.dma_start(wfcT_sb[:, o, :], wfcT[o * P:(o + 1) * P, :])

        def dma_wproj(o):
            nc.sync.dma_start(wprojT_sb[:, o, :], wprojT[o * P:(o + 1) * P, :])

        def dma_resid(tt):
            nc.sync.dma_start(resid_sb[:, tt, :], resid[tt * P:(tt + 1) * P, :])

        # SP-queue issue order = consumption order: the first sub-stream
        # needs row tiles 0-3, wfc chunks progressively and wproj from block
        # ~4; tiles 4-7 arrive while sub-stream 0 executes.
        for tt in range(4):
            dma_resid(tt)
        dma_wfc(0); dma_wfc(1)
        dma_wfc(2); dma_wfc(3)
        if CONFIG["r45_early"]:
            for o in range(4):
                dma_wproj(o)
            dma_resid(4)
            dma_wproj(4); dma_wproj(5)
            dma_resid(5)
            dma_wproj(6); dma_wproj(7)
        else:
            for o in range(8):
                dma_wproj(o)
            dma_resid(4); dma_resid(5)
        dma_wfc(4); dma_wfc(5)
        dma_resid(6); dma_resid(7)
        dma_wfc(6); dma_wfc(7)

        pT_dummy = None
        if variant != 'full':
            pT_dummy = singles.tile([P, 512], BF16)
            nc.sync.dma_start(pT_dummy[:], wfcT[0:P, 0:512])
            nc.sync.dma_start(xT_sb[:], wfcT.rearrange("(o p) i -> p o i", p=P)
                              [:, :, 0:ROWS])
            nc.vector.memset(out_sb[:], 0.0)

        if reps == 1:
            _phase_abc(nc, tc, work, rlpool, ptpool, psA, psO, psT, ident_bf,
                       resid, out, wfcT_sb, wprojT_sb, xT_sb, resid_sb,
                       out_sb, eps_t, sb_sb, 0, variant, pT_dummy, has_bias)
        else:
            hint = (mybir.EngineType.PE, mybir.EngineType.Activation,
                    mybir.EngineType.DVE, mybir.EngineType.SP,
                    mybir.EngineType.Pool)
            with tc.For_i(0, reps, 1, hint_engines=hint):
                _phase_abc(nc, tc, work, rlpool, ptpool, psA, psO, psT,
                           ident_bf, resid, out, wfcT_sb, wprojT_sb, xT_sb,
                           resid_sb, out_sb, eps_t, sb_sb, 0, variant,
                           pT_dummy, has_bias)


def _phase_abc(nc, tc, work, rlpool, ptpool, psA, psO, psT, ident_bf,
               resid, out, wfcT_sb, wprojT_sb, xT_sb, resid_sb, out_sb,
               eps_t, sb_sb, rep, variant, pT_dummy, has_bias):
    full = variant == 'full'
    pat = CONFIG["relu_pat"]
    lookahead = CONFIG["lookahead"]
    blk_idx = [0]

    # sub-streams in emission order: (head, token-half)
    streams = [(h, tch) for h in range(QH) for tch in range(2)]
    po = {}      # (h, tch) -> [po_dd0, po_dd1]

    def relu_sq(ps, hb, h, cc, ich):
        """pT = relu(ps + bias)^2 in bf16, engines per pattern schedule."""
        kind = pat[hb % len(pat)]
        rl = rlpool.tile([P, 512], BF16, name=f"rl_{rep}_{hb}", tag="rl")
        pT = ptpool.tile([P, 512], BF16, name=f"pT_{rep}_{hb}", tag="pT")
        if has_bias:
            bias = sb_sb[:, h * 32 + cc * 8 + ich:h * 32 + cc * 8 + ich + 1]
            nc.scalar.activation(rl[:], ps[:],
                                 mybir.ActivationFunctionType.Relu,
                                 bias=bias, scale=1.0)
            if kind in ('Z',):
                nc.vector.tensor_mul(out=pT[:], in0=rl[:], in1=rl[:])
            elif kind in ('Y', 'V'):
                nc.gpsimd.tensor_mul(out=pT[:], in0=rl[:], in1=rl[:])
            else:
                nc.scalar.activation(pT[:], rl[:],
                                     mybir.ActivationFunctionType.Square)
            return pT
        if kind == 'S':
            # split: ACT owns relu+square of half 0, DVE of half 1 — no
            # cross-engine dep inside the block, psA releases ~2x sooner
            nc.scalar.activation(rl[:, 0:256], ps[:, 0:256],
                                 mybir.ActivationFunctionType.Relu)
            nc.scalar.activation(pT[:, 0:256], rl[:, 0:256],
                                 mybir.ActivationFunctionType.Square)
            nc.vector.tensor_scalar_max(out=rl[:, 256:512], in0=ps[:, 256:512],
                                        scalar1=0.0)
            nc.vector.tensor_mul(out=pT[:, 256:512], in0=rl[:, 256:512],
                                 in1=rl[:, 256:512])
        elif kind == 'Z':
            nc.scalar.activation(rl[:], ps[:],
                                 mybir.ActivationFunctionType.Relu)
            nc.vector.tensor_mul(out=pT[:], in0=rl[:], in1=rl[:])
        elif kind == 'Y':
            nc.scalar.activation(rl[:], ps[:],
                                 mybir.ActivationFunctionType.Relu)
            nc.gpsimd.tensor_mul(out=pT[:], in0=rl[:], in1=rl[:])
        elif kind == 'V':
            nc.vector.tensor_scalar_max(out=rl[:], in0=ps[:], scalar1=0.0)
            nc.gpsimd.tensor_mul(out=pT[:], in0=rl[:], in1=rl[:])
        else:  # W
            nc.vector.tensor_scalar_max(out=rl[:], in0=ps[:], scalar1=0.0)
            nc.scalar.activation(pT[:], rl[:],
                                 mybir.ActivationFunctionType.Square)
        return pT

    def mm1(h, tch, cc, ich, hb):
        ps = psA.tile([P, 512], F32, name=f"psa_{rep}_{hb}", tag="psa")
        tsl = slice(tch * 512, (tch + 1) * 512)
        isl = slice(ich * P, (ich + 1) * P)
        nc.tensor.matmul(ps[:], wfcT_sb[:, cc * 2 + 0, isl],
                         xT_sb[:, h * 2 + 0, tsl], start=True, stop=False)
        nc.tensor.matmul(ps[:], wfcT_sb[:, cc * 2 + 1, isl],
                         xT_sb[:, h * 2 + 1, tsl], start=False, stop=True)
        return ps

    def mm2(h, tch, cc, ich, pT):
        first = (cc == 0 and ich == 0)
        last = (cc == NCC - 1 and ich == 7)
        for dd in range(2):
            wsl = slice(cc * D + dd * P, cc * D + (dd + 1) * P)
            nc.tensor.matmul(po[(h, tch)][dd][:],
                             wprojT_sb[:, ich, wsl], pT[:],
                             start=first, stop=last)

    def epilogue(h, tch):
        if variant not in ('full', 'b_only'):
            return
        for dd in range(2):
            oc = work.tile([P, 512], BF16, name=f"oc_{rep}_{h}_{tch}_{dd}",
                           tag="oc")
            nc.scalar.activation(oc[:], po[(h, tch)][dd][:],
                                 mybir.ActivationFunctionType.Identity)
            ot = work.tile([P, 4, P], BF16, name=f"ot_{rep}_{h}_{tch}_{dd}",
                           tag="ot")
            nc.sync.dma_start_transpose(ot[:], oc[:])
            csl = slice(h * D + dd * P, h * D + (dd + 1) * P)
            # alternate add engines so the two halves run in parallel
            eng = nc.vector if dd == 0 else nc.gpsimd
            eng.tensor_add(out=out_sb[:, tch * 4:(tch + 1) * 4, csl],
                           in0=out_sb[:, tch * 4:(tch + 1) * 4, csl],
                           in1=ot[:])
            for k in range(4):
                tt = tch * 4 + k
                nc.sync.dma_start(out[tt * P:(tt + 1) * P, csl],
                                  out_sb[:, tt, csl])

    # ---- emission ----------------------------------------------------------
    if full:
        for tt in range(4):
            _phase_a_tile(nc, work, psT, resid_sb, out_sb, xT_sb, ident_bf,
                          eps_t, tt, rep, psT is psA)

    pending = []       # (h, tch, cc, ich, pT) awaiting mm2
    prev_stream = [None]

    for si, (h, tch) in enumerate(streams):
        po[(h, tch)] = [psO.tile([P, 512], F32,
                                 name=f"po_{rep}_{h}_{tch}_{dd}", tag="po")
                        for dd in range(2)]
        for cc in range(NCC):
            for ich in range(8):
                hb = blk_idx[0]
                blk_idx[0] += 1
                nblk = cc * 8 + ich
                if nblk == CONFIG["epi_at"] and prev_stream[0] is not None:
                    epilogue(*prev_stream[0])
                    prev_stream[0] = None
                if si == 0 and full and nblk in CONFIG["pa2_at"]:
                    tt = 4 + CONFIG["pa2_at"].index(nblk)
                    _phase_a_tile(nc, work, psT, resid_sb, out_sb, xT_sb,
                                  ident_bf, eps_t, tt, rep, psT is psA,
                                  use_dma=CONFIG["pa2_dma"])
                ps = mm1(h, tch, cc, ich, hb)
                if variant == 'mm1_only':
                    continue
                if variant == 'mm_only':
                    mm2(h, tch, cc, ich, pT_dummy)
                    continue
                pT = relu_sq(ps, hb, h, cc, ich)
                pending.append((h, tch, cc, ich, pT))
                if len(pending) > lookahead:
                    a = pending.pop(0)
                    mm2(a[0], a[1], a[2], a[3], a[4])
        # the pipeline continues across sub-streams: this stream's last mm2s
        # drain behind the next stream's mm1s, and its epilogue (ACT/DMA/Pool
        # only) is emitted epi_at blocks into the next stream
        prev_stream[0] = (h, tch)
    for a in pending:
        mm2(a[0], a[1], a[2], a[3], a[4])
    if prev_stream[0] is not None:
        epilogue(*prev_stream[0])


def build_nc(reps=1, variant='full', has_bias=False):
    key = (reps, variant, has_bias, str(sorted(CONFIG.items())))
    if key in _NC_CACHE:
        return _NC_CACHE[key]
    nc = bacc.Bacc("TRN2", target_bir_lowering=False, debug=False,
                   num_devices=N_CORES)
    resid = nc.dram_tensor("residual", [ROWS, C], F32, kind="ExternalInput").ap()
    wfcT = nc.dram_tensor("w_fcT", [C, C], BF16, kind="ExternalInput").ap()
    wprojT = nc.dram_tensor("w_projT", [C, C], BF16, kind="ExternalInput").ap()
    sbias = nc.dram_tensor("s_bias", [128 * P], F32, kind="ExternalInput").ap() \
        if True else None
    out = nc.dram_tensor("out", [ROWS, C], F32, kind="ExternalOutput").ap()
    with tile.TileContext(nc) as tc:
        _build_body(tc, resid, wfcT, wprojT, sbias, out, reps, variant,
                    has_bias)
    nc.compile()
    _NC_CACHE[key] = nc
    return nc


def _in_maps(residual, w_fc, w_proj, ln_g, ln_b):
    import ml_dtypes
    resid2d = np.ascontiguousarray(residual.reshape(-1, C))
    # fold ln_g into the keys: scores use x_hat * g as the query
    wfcT = np.ascontiguousarray((w_fc * np.asarray(ln_g)[None, :]).T
                                ).astype(ml_dtypes.bfloat16)
    wprojT = np.ascontiguousarray(w_proj.T).astype(ml_dtypes.bfloat16)
    # per-key score bias from ln_b: sb[h, s=(i,cc)] = sum_d b[h*D+d] k[s,d]
    ln_b = np.asarray(ln_b)
    k = w_fc.reshape(-1, D)                           # (S, D), s = i*4+cc
    sb = ln_b.reshape(QH, D) @ k.T                    # (QH, S)
    # device layout: sb_dev[p, h*32+cc*8+ich] with i = ich*128+p
    s_idx = (np.arange(4096).reshape(-1, 4))          # [i, cc] -> s
    sb_dev = np.zeros((P, 128), np.float32)
    for h in range(QH):
        for cc in range(NCC):
            for ich in range(8):
                i = np.arange(ich * P, (ich + 1) * P)
                sb_dev[:, h * 32 + cc * 8 + ich] = sb[h, s_idx[i, cc]]
    sb_flat = np.ascontiguousarray(sb_dev.T.reshape(-1))  # (k p) -> p k
    return [
        {"residual": resid2d[i * ROWS:(i + 1) * ROWS],
         "w_fcT": wfcT, "w_projT": wprojT, "s_bias": sb_flat}
        for i in range(N_CORES)
    ]


def run_on_cores(inputs, reps=1):
    has_bias = bool(np.any(np.asarray(inputs["ln_b"])))
    nc = build_nc(reps, 'full', has_bias)
    in_maps = _in_maps(**inputs)
    return run_bass_kernel_spmd(nc, in_maps, core_ids=list(range(N_CORES)))


def kernel(residual, w_fc, w_proj, ln_g, ln_b):
    B, T, Cx = residual.shape
    res = run_on_cores(dict(residual=residual, w_fc=w_fc, w_proj=w_proj,
                            ln_g=ln_g, ln_b=ln_b))
    out = np.concatenate([r["out"] for r in res.results], axis=0)
    return out.reshape(B, T, Cx).astype(np.float32)


# revision 35
# speedup vs baseline: 1.0210x; 1.0161x over previous
"""Trainium2 Bass kernel for nn_MLPMHA (sparse_attention / squared-ReLU MLP-MHA).

Reference computation (B=4, T=2048, C=1024, QH=4, D=256, S=4C=4096):
    x   = layernorm(residual) * g + b
    q_h = x[:, h*D:(h+1)*D]                     per head h
    k   = w_fc.reshape(S, D)                    keys   (shared across heads)
    v   = w_proj.T.reshape(S, D)                values (shared across heads)
    out = residual + concat_h( relu(q_h @ k.T)^2 @ v )

Sharding: pure data parallel over the 8192 = B*T token rows; each of the 8
cores processes 1024 rows with full weights resident in SBUF.

v2 design (vs the fp32r v1 baseline; 443us recorded -> ~301us measured):
  * ln_g is folded into w_fc on the host (wfcT_eff = (w_fc * g).T); a nonzero
    ln_b adds a per-key score bias sb[h,s] (host precomputed) applied in the
    relu stage. With that, x_hat needs no per-channel affine after the
    transpose into [channel, token] layout.
  * All matmul operands are bf16 (same 1 cycle/row as fp32r on the PE, but
    half the SBUF/DMA traffic and 2x faster PE transposes; measured
    end-to-end rel err ~4e-3 vs the 2e-2 tolerance).
  * relu^2 splits each [128,512] PSUM tile in half: ACT runs relu+square on
    one half, DVE on the other (pattern "S") — no cross-engine dependency
    inside a block and the psA bank releases ~2x sooner.
  * Phase B is split into 8 sub-streams (head x token-half); the first only
    needs row tiles 0-3 transposed, so the PE starts ~9us in instead of
    ~31us. Tiles 4-7 run their LayerNorm mid-sub-stream-0 and are transposed
    by the DMA XBAR (dma_start_transpose, exact for 2-byte dtypes) on the
    then-idle SP ring; tiles 0-3 use PE transposes (PE is idle at the head).
  * The mm1->relu^2->mm2 chain is software-pipelined with a global pending
    queue that crosses sub-stream boundaries (lookahead blocks); epilogues
    (PSUM -> bf16 -> DMA-transpose -> DVE/Pool add into residual-initialised
    out_sb -> per-column DMA store) are emitted epi_at (> lookahead!) blocks
    into the next sub-stream and never touch the PE.
  * Weight/residual DMAs are issued in consumption order so the first matmuls
    wait only on the first ~1.5MB, not the full 8MB of inputs.

Hardware-measured notes: matmul instruction streams cost ~52ns/mm extra at
kernel-scale loop bodies (instruction fetch) plus ~100ns per cross-engine
semaphore wait; deep buffer rotation (psA=4, pt/rl pools) recovers most of
the latter. A 1024-wide matmul output is illegal (s3d3_mm_num_elements: one
PSUM bank max). GPSIMD/Pool has no PSUM port, and scalar_tensor_tensor may
read at most one PSUM operand.
"""

import numpy as np

import concourse.bass as bass
import concourse.tile as tile
from concourse import mybir, bacc
from concourse.bass_utils import run_bass_kernel_spmd

P = 128
C = 1024
D = 256
QH = 4
NCC = 4          # column chunks of w_fc (S = NCC * C kv entries)
N_CORES = 8
ROWS = 1024      # token rows per core (8192 / 8)
NT = ROWS // P   # 8 row tiles per core
EPS = 1e-5

F32 = mybir.dt.float32
BF16 = mybir.dt.bfloat16

_NC_CACHE = {}

CONFIG = {
    "lookahead": 3,        # mm2 software-pipeline depth behind mm1
    "relu_pat": "S",         # relu^2 engine pattern, cycled per half-block:
                             # S: tile split in half, ACT owns relu+square of
                             #    one half and DVE the other (fastest)
                             # Z: ACT relu + DVE square   Y: ACT relu + Pool sq
                             # V: DVE relu + Pool square  W: DVE relu + ACT sq
    "pools": (3, 5, 0),    # psA, psO bufs (psT=0: transposes share psA)
    "pt_bufs": 7,          # pT half-tiles in flight
    "rl_bufs": 5,
    "pa2_dma": True,       # transpose tiles 4-7 via the SP DMA ring (idle
                           # mid-stream) instead of PE
    "r45_early": False,    # interleave r4/r5 into the wproj chunk stream
    "epi_at": 5,           # block in sub-stream k where sub-stream k-1's
                           # epilogue is emitted; MUST be > lookahead, or the
                           # epilogue reads po before its last mm2s are
                           # emitted
    "pa2_at": (10, 14, 18, 22),  # blocks of sub-stream 0 where row tiles
                                 # 4-7 run their phase A
}


def _ln_tile(nc, work, resid_sb, tt, eps_t, rep):
    """LayerNorm stats for row tile tt; returns bf16 x_hat tile."""
    stats = work.tile([P, 2, 6], F32, name=f"st_{rep}_{tt}", tag="stats")
    nc.vector.bn_stats(stats[:, 0, :], resid_sb[:, tt, 0:512])
    nc.vector.bn_stats(stats[:, 1, :], resid_sb[:, tt, 512:1024])
    mv = work.tile([P, 2], F32, name=f"mv_{rep}_{tt}", tag="mv")
    nc.vector.bn_aggr(mv[:], stats[:])
    nc.scalar.activation(mv[:, 1:2], mv[:, 1:2],
                         mybir.ActivationFunctionType.Sqrt,
                         bias=eps_t[:], scale=1.0)
    nc.vector.reciprocal(mv[:, 1:2], mv[:, 1:2])
    nmr = work.tile([P, 1], F32, name=f"nmr_{rep}_{tt}", tag="nmr")
    nc.vector.tensor_scalar(out=nmr[:], in0=mv[:, 0:1],
                            scalar1=mv[:, 1:2], scalar2=-1.0,
                            op0=mybir.AluOpType.mult,
                            op1=mybir.AluOpType.mult)
    xn = work.tile([P, C], BF16, name=f"xn_{rep}_{tt}", tag="xn")
    nc.scalar.activation(xn[:], resid_sb[:, tt, :],
                         mybir.ActivationFunctionType.Identity,
                         bias=nmr[:], scale=mv[:, 1:2])
    return xn


def _phase_a_tile(nc, work, psT, resid_sb, out_sb, xT_sb, ident_bf, eps_t,
                  tt, rep, psT_is_psA=False, use_dma=False):
    xn = _ln_tile(nc, work, resid_sb, tt, eps_t, rep)
    if use_dma:
        # XBAR transpose on the SP hwdge ring (idle mid-stream):
        # xT_sb[p, o, tt*P+t] = xn[t, o*P+p]
        nc.sync.dma_start_transpose(xT_sb[:, :, tt * P:(tt + 1) * P], xn[:])
    else:
        # PE transpose in bf16 (1 cycle/row): with ln_g folded into the
        # weights there is no affine, just a PSUM copy-back on ACT.
        for och in range(8):
            _ptag = "psa" if psT_is_psA else "pst"
            pst = psT.tile([P, P], BF16, name=f"psx_{rep}_{tt}_{och}",
                           tag=_ptag)
            nc.tensor.transpose(pst[:], xn[:, och * P:(och + 1) * P],
                                ident_bf[:])
            nc.scalar.activation(xT_sb[:, och, tt * P:(tt + 1) * P], pst[:],
                                 mybir.ActivationFunctionType.Identity)
    # out starts as the residual; head outputs accumulate into it
    nc.gpsimd.tensor_copy(out=out_sb[:, tt, :], in_=resid_sb[:, tt, :])


def _build_body(tc, resid, wfcT, wprojT, sbias, out, reps, variant, has_bias):  # noqa: C901
    nc = tc.nc
    import contextlib
    ctx = contextlib.ExitStack()
    with ctx:
        singles = ctx.enter_context(tc.tile_pool(name="singles", bufs=1))
        work = ctx.enter_context(tc.tile_pool(name="work", bufs=3))
        rlpool = ctx.enter_context(tc.tile_pool(name="rlpool",
                                                bufs=CONFIG["rl_bufs"]))
        ptpool = ctx.enter_context(tc.tile_pool(name="ptpool",
                                                bufs=CONFIG["pt_bufs"]))
        psA = ctx.enter_context(tc.tile_pool(name="psA", bufs=CONFIG["pools"][0],
                                             space="PSUM"))
        psO = ctx.enter_context(tc.tile_pool(name="psO", bufs=CONFIG["pools"][1],
                                             space="PSUM"))
        if CONFIG["pools"][2]:
            psT = ctx.enter_context(tc.tile_pool(name="psT",
                                                 bufs=CONFIG["pools"][2],
                                                 space="PSUM"))
        else:
            psT = psA  # transposes rotate through the psA slots

        # ---- resident tensors, DMA'd in consumption order ----------------
        wfcT_sb = singles.tile([P, 8, C], BF16)
        wprojT_sb = singles.tile([P, 8, C], BF16)
        xT_sb = singles.tile([P, 8, ROWS], BF16)
        resid_sb = singles.tile([P, NT, C], F32)
        out_sb = singles.tile([P, NT, C], F32)
        eps_t = singles.tile([P, 1], F32)
        nc.vector.memset(eps_t[:], EPS)
        ident_bf = singles.tile([P, P], BF16)
        from concourse.masks import make_identity
        make_identity(nc, ident_bf[:])
        sb_sb = None
        if has_bias:
            sb_sb = singles.tile([P, 128], F32)
            nc.sync.dma_start(sb_sb[:], sbias.rearrange("(k p) -> p k", p=P))

        def dma_wfc(o):
            nc.sync.dma_start(wfcT_sb[:, o, :], wfcT[o * P:(o + 1) * P, :])

        def dma_wproj(o):
            nc.sync.dma_start(wprojT_sb[:, o, :], wprojT[o * P:(o + 1) * P, :])

        def dma_resid(tt, split=False):
            if split:
                # halves let bn_stats start as soon as the first 512 cols land
                nc.sync.dma_start(resid_sb[:, tt, 0:512],
                                  resid[tt * P:(tt + 1) * P, 0:512])
                nc.sync.dma_start(resid_sb[:, tt, 512:1024],
                                  resid[tt * P:(tt + 1) * P, 512:1024])
            else:
                nc.sync.dma_start(resid_sb[:, tt, :],
                                  resid[tt * P:(tt + 1) * P, :])

        # SP-queue issue order = consumption order: the first sub-stream
        # needs row tiles 0-3, wfc chunks progressively and wproj from block
        # ~4; tiles 4-7 arrive while sub-stream 0 executes.
        for tt in range(4):
            dma_resid(tt, split=True)
        dma_wfc(0); dma_wfc(1)
        dma_wfc(2); dma_wfc(3)
        if CONFIG["r45_early"]:
            for o in range(4):
                dma_wproj(o)
            dma_resid(4)
            dma_wproj(4); dma_wproj(5)
            dma_resid(5)
            dma_wproj(6); dma_wproj(7)
        else:
            for o in range(8):
                dma_wproj(o)
            dma_resid(4); dma_resid(5)
        dma_wfc(4); dma_wfc(5)
        dma_resid(6); dma_resid(7)
        dma_wfc(6); dma_wfc(7)

        pT_dummy = None
        if variant != 'full':
            pT_dummy = singles.tile([P, 512], BF16)
            nc.sync.dma_start(pT_dummy[:], wfcT[0:P, 0:512])
            nc.sync.dma_start(xT_sb[:], wfcT.rearrange("(o p) i -> p o i", p=P)
                              [:, :, 0:ROWS])
            nc.vector.memset(out_sb[:], 0.0)

        if reps == 1:
            _phase_abc(nc, tc, work, rlpool, ptpool, psA, psO, psT, ident_bf,
                       resid, out, wfcT_sb, wprojT_sb, xT_sb, resid_sb,
                       out_sb, eps_t, sb_sb, 0, variant, pT_dummy, has_bias)
        else:
            hint = (mybir.EngineType.PE, mybir.EngineType.Activation,
                    mybir.EngineType.DVE, mybir.EngineType.SP,
                    mybir.EngineType.Pool)
            with tc.For_i(0, reps, 1, hint_engines=hint):
                _phase_abc(nc, tc, work, rlpool, ptpool, psA, psO, psT,
                           ident_bf, resid, out, wfcT_sb, wprojT_sb, xT_sb,
                           resid_sb, out_sb, eps_t, sb_sb, 0, variant,
                           pT_dummy, has_bias)


def _phase_abc(nc, tc, work, rlpool, ptpool, psA, psO, psT, ident_bf,
               resid, out, wfcT_sb, wprojT_sb, xT_sb, resid_sb, out_sb,
               eps_t, sb_sb, rep, variant, pT_dummy, has_bias):
    full = variant == 'full'
    pat = CONFIG["relu_pat"]
    lookahead = CONFIG["lookahead"]
    blk_idx = [0]

    # sub-streams in emission order: (head, token-half)
    streams = [(h, tch) for h in range(QH) for tch in range(2)]
    po = {}      # (h, tch) -> [po_dd0, po_dd1]

    def relu_sq(ps, hb, h, cc, ich):
        """pT = relu(ps + bias)^2 in bf16, engines per pattern schedule."""
        kind = pat[hb % len(pat)]
        rl = rlpool.tile([P, 512], BF16, name=f"rl_{rep}_{hb}", tag="rl")
        pT = ptpool.tile([P, 512], BF16, name=f"pT_{rep}_{hb}", tag="pT")
        if has_bias:
            bias = sb_sb[:, h * 32 + cc * 8 + ich:h * 32 + cc * 8 + ich + 1]
            nc.scalar.activation(rl[:], ps[:],
                                 mybir.ActivationFunctionType.Relu,
                                 bias=bias, scale=1.0)
            if kind in ('Z',):
                nc.vector.tensor_mul(out=pT[:], in0=rl[:], in1=rl[:])
            elif kind in ('Y', 'V'):
                nc.gpsimd.tensor_mul(out=pT[:], in0=rl[:], in1=rl[:])
            else:
                nc.scalar.activation(pT[:], rl[:],
                                     mybir.ActivationFunctionType.Square)
            return pT
        if kind == 'S':
            # split: ACT owns relu+square of half 0, DVE of half 1 — no
            # cross-engine dep inside the block, psA releases ~2x sooner
            nc.scalar.activation(rl[:, 0:256], ps[:, 0:256],
                                 mybir.ActivationFunctionType.Relu)
            nc.scalar.activation(pT[:, 0:256], rl[:, 0:256],
                                 mybir.ActivationFunctionType.Square)
            nc.vector.tensor_scalar_max(out=rl[:, 256:512], in0=ps[:, 256:512],
                                        scalar1=0.0)
            nc.vector.tensor_mul(out=pT[:, 256:512], in0=rl[:, 256:512],
                                 in1=rl[:, 256:512])
        elif kind == 'Z':
            nc.scalar.activation(rl[:], ps[:],
                                 mybir.ActivationFunctionType.Relu)
            nc.vector.tensor_mul(out=pT[:], in0=rl[:], in1=rl[:])
        elif kind == 'Y':
            nc.scalar.activation(rl[:], ps[:],
                                 mybir.ActivationFunctionType.Relu)
            nc.gpsimd.tensor_mul(out=pT[:], in0=rl[:], in1=rl[:])
        elif kind == 'V':
            nc.vector.tensor_scalar_max(out=rl[:], in0=ps[:], scalar1=0.0)
            nc.gpsimd.tensor_mul(out=pT[:], in0=rl[:], in1=rl[:])
        else:  # W
            nc.vector.tensor_scalar_max(out=rl[:], in0=ps[:], scalar1=0.0)
            nc.scalar.activation(pT[:], rl[:],
                                 mybir.ActivationFunctionType.Square)
        return pT

    def mm1(h, tch, cc, ich, hb):
        ps = psA.tile([P, 512], F32, name=f"psa_{rep}_{hb}", tag="psa")
        tsl = slice(tch * 512, (tch + 1) * 512)
        isl = slice(ich * P, (ich + 1) * P)
        nc.tensor.matmul(ps[:], wfcT_sb[:, cc * 2 + 0, isl],
                         xT_sb[:, h * 2 + 0, tsl], start=True, stop=False)
        nc.tensor.matmul(ps[:], wfcT_sb[:, cc * 2 + 1, isl],
                         xT_sb[:, h * 2 + 1, tsl], start=False, stop=True)
        return ps

    def mm2(h, tch, cc, ich, pT):
        first = (cc == 0 and ich == 0)
        last = (cc == NCC - 1 and ich == 7)
        for dd in range(2):
            wsl = slice(cc * D + dd * P, cc * D + (dd + 1) * P)
            nc.tensor.matmul(po[(h, tch)][dd][:],
                             wprojT_sb[:, ich, wsl], pT[:],
                             start=first, stop=last)

    def epilogue(h, tch):
        if variant not in ('full', 'b_only'):
            return
        for dd in range(2):
            oc = work.tile([P, 512], BF16, name=f"oc_{rep}_{h}_{tch}_{dd}",
                           tag="oc")
            nc.scalar.activation(oc[:], po[(h, tch)][dd][:],
                                 mybir.ActivationFunctionType.Identity)
            ot = work.tile([P, 4, P], BF16, name=f"ot_{rep}_{h}_{tch}_{dd}",
                           tag="ot")
            nc.sync.dma_start_transpose(ot[:], oc[:])
            csl = slice(h * D + dd * P, h * D + (dd + 1) * P)
            # alternate add engines so the two halves run in parallel
            eng = nc.vector if dd == 0 else nc.gpsimd
            eng.tensor_add(out=out_sb[:, tch * 4:(tch + 1) * 4, csl],
                           in0=out_sb[:, tch * 4:(tch + 1) * 4, csl],
                           in1=ot[:])
            for k in range(4):
                tt = tch * 4 + k
                nc.sync.dma_start(out[tt * P:(tt + 1) * P, csl],
                                  out_sb[:, tt, csl])

    # ---- emission ----------------------------------------------------------
    if full:
        for tt in range(4):
            _phase_a_tile(nc, work, psT, resid_sb, out_sb, xT_sb, ident_bf,
                          eps_t, tt, rep, psT is psA)

    pending = []       # (h, tch, cc, ich, pT) awaiting mm2
    prev_stream = [None]

    for si, (h, tch) in enumerate(streams):
        po[(h, tch)] = [psO.tile([P, 512], F32,
                                 name=f"po_{rep}_{h}_{tch}_{dd}", tag="po")
                        for dd in range(2)]
        for cc in range(NCC):
            for ich in range(8):
                hb = blk_idx[0]
                blk_idx[0] += 1
                nblk = cc * 8 + ich
                if nblk == CONFIG["epi_at"] and prev_stream[0] is not None:
                    epilogue(*prev_stream[0])
                    prev_stream[0] = None
                if si == 0 and full and nblk in CONFIG["pa2_at"]:
                    tt = 4 + CONFIG["pa2_at"].index(nblk)
                    _phase_a_tile(nc, work, psT, resid_sb, out_sb, xT_sb,
                                  ident_bf, eps_t, tt, rep, psT is psA,
                                  use_dma=CONFIG["pa2_dma"])
                ps = mm1(h, tch, cc, ich, hb)
                if variant == 'mm1_only':
                    continue
                if variant == 'mm_only':
                    mm2(h, tch, cc, ich, pT_dummy)
                    continue
                pT = relu_sq(ps, hb, h, cc, ich)
                pending.append((h, tch, cc, ich, pT))
                if len(pending) > lookahead:
                    a = pending.pop(0)
                    mm2(a[0], a[1], a[2], a[3], a[4])
        # the pipeline continues across sub-streams: this stream's last mm2s
        # drain behind the next stream's mm1s, and its epilogue (ACT/DMA/Pool
        # only) is emitted epi_at blocks into the next stream
        prev_stream[0] = (h, tch)
    for a in pending:
        mm2(a[0], a[1], a[2], a[3], a[4])
    if prev_stream[0] is not None:
        epilogue(*prev_stream[0])


def build_nc(reps=1, variant='full', has_bias=False):
    key = (reps, variant, has_bias, str(sorted(CONFIG.items())))
    if key in _NC_CACHE:
        return _NC_CACHE[key]
    nc = bacc.Bacc("TRN2", target_bir_lowering=False, debug=False,
                   num_devices=N_CORES)
    resid = nc.dram_tensor("residual", [ROWS, C], F32, kind="ExternalInput").ap()
    wfcT = nc.dram_tensor("w_fcT", [C, C], BF16, kind="ExternalInput").ap()
    wprojT = nc.dram_tensor("w_projT", [C, C], BF16, kind="ExternalInput").ap()
    sbias = nc.dram_tensor("s_bias", [128 * P], F32, kind="ExternalInput").ap() \
        if True else None
    out = nc.dram_tensor("out", [ROWS, C], F32, kind="ExternalOutput").ap()
    with tile.TileContext(nc) as tc:
        _build_body(tc, resid, wfcT, wprojT, sbias, out, reps, variant,
                    has_bias)
    nc.compile()
    _NC_CACHE[key] = nc
    return nc


def _in_maps(residual, w_fc, w_proj, ln_g, ln_b):
    import ml_dtypes
    resid2d = np.ascontiguousarray(residual.reshape(-1, C))
    # fold ln_g into the keys: scores use x_hat * g as the query
    wfcT = np.ascontiguousarray((w_fc * np.asarray(ln_g)[None, :]).T
                                ).astype(ml_dtypes.bfloat16)
    wprojT = np.ascontiguousarray(w_proj.T).astype(ml_dtypes.bfloat16)
    # per-key score bias from ln_b: sb[h, s=(i,cc)] = sum_d b[h*D+d] k[s,d]
    ln_b = np.asarray(ln_b)
    k = w_fc.reshape(-1, D)                           # (S, D), s = i*4+cc
    sb = ln_b.reshape(QH, D) @ k.T                    # (QH, S)
    # device layout: sb_dev[p, h*32+cc*8+ich] with i = ich*128+p
    s_idx = (np.arange(4096).reshape(-1, 4))          # [i, cc] -> s
    sb_dev = np.zeros((P, 128), np.float32)
    for h in range(QH):
        for cc in range(NCC):
            for ich in range(8):
                i = np.arange(ich * P, (ich + 1) * P)
                sb_dev[:, h * 32 + cc * 8 + ich] = sb[h, s_idx[i, cc]]
    sb_flat = np.ascontiguousarray(sb_dev.T.reshape(-1))  # (k p) -> p k
    return [
        {"residual": resid2d[i * ROWS:(i + 1) * ROWS],
         "w_fcT": wfcT, "w_projT": wprojT, "s_bias": sb_flat}
        for i in range(N_CORES)
    ]


def run_on_cores(inputs, reps=1):
    has_bias = bool(np.any(np.asarray(inputs["ln_b"])))
    nc = build_nc(reps, 'full', has_bias)
    in_maps = _in_maps(**inputs)
    return run_bass_kernel_spmd(nc, in_maps, core_ids=list(range(N_CORES)))


def kernel(residual, w_fc, w_proj, ln_g, ln_b):
    B, T, Cx = residual.shape
    res = run_on_cores(dict(residual=residual, w_fc=w_fc, w_proj=w_proj,
                            ln_g=ln_g, ln_b=ln_b))
    out = np.concatenate([r["out"] for r in res.results], axis=0)
    return out.reshape(B, T, Cx).astype(np.float32)
